# revision 8
# baseline (speedup 1.0000x reference)
"""Non-local block (B=4, C_in=256, C_int=128, C_out=256, N=T*H*W=4096) on 8
Trainium2 NeuronCores.

Sharding: data-parallel over batch (4 batches) x query-halves (2) = 8 cores.
Each core holds one batch's full x (for keys/values); the host rotates each
core's columns so its 2048 queries are always columns 0:2048 (attention is
permutation-invariant over keys). Per core: theta/phi/g projections, the
[2048q x 4096k] attention with softmax (keys on partitions), and the output
projection for its query half. Host gathers the 8 [256, 2048] slices.

v2 engine layout:
  PE:   scores + y + projections + bf16 gT transposes + denominator
        broadcast (ones matmul).  Single PSUM scope for the whole kernel
        (scores ring 2x[128,1024]=4 banks, y 1x[128,1024]=2, proj/piece ring
        2x[128,512]=2) so there is no pool-boundary drain between the two
        query groups.
  Act:  pure exp stream (plus final-group y evac).  Table pre-loaded via a
        dummy exp during the DMA wait.
  DVE:  PSUM evacuations (theta/phi f32r, g bf16, gT copy, y f32r), the
        high columns of the softmax denominator accumulation, the epilogue
        (cast/recip/scale), and a Schraudolph approximate-exp share of the
        late group-1 tiles (one tensor_scalar: bf16 bits = s*184.665+16250.5
        converted to uint16).
  Pool (gpsimd): low columns of the denominator accumulation
        (SBUF-only; GPSIMD cannot touch PSUM).
  DMA:  staged; the x tail + cold constants are gated behind the first
        theta evacuation so the startup-critical cpak_hot+x0 transfers get
        full HBM bandwidth.
"""

import sys
import types

import numpy as np

import concourse.bacc as bacc
import concourse.mybir as mybir
import concourse.tile as tile
from concourse.bass_utils import run_bass_kernel_spmd
from concourse.tile import add_dep_helper


def _install_ntff_hook():
    try:
        import antenv.axon_hooks  # noqa: F401
        return
    except ImportError:
        pass
    try:
        from trn_agent_boot.trn_boot import _ntff_profile_via_ctypes

        hook = _ntff_profile_via_ctypes("/opt/axon/libaxon_pjrt.so")
    except Exception:
        hook = None
    mod = types.ModuleType("antenv.axon_hooks")
    mod.get_axon_ntff_profile_hook = lambda: hook
    mod.set_axon_ntff_profile_hook = lambda h: None
    sys.modules["antenv.axon_hooks"] = mod


_install_ntff_hook()

F32 = mybir.dt.float32
F32R = mybir.dt.float32r
BF16 = mybir.dt.bfloat16
U16 = mybir.dt.uint16
AF = mybir.ActivationFunctionType
OP = mybir.AluOpType

P = 128
CI = 256  # input channels (2 chunks of 128)
CINT = 128  # intermediate channels
CO = 256  # output channels (2 blocks of 128)
N = 4096  # key/value positions (32 blocks of 128)
Q = 2048  # queries per core
B, T, H, W = 4, 4, 32, 32
NKB = N // P  # 32 key blocks

MM_DT = F32R

# Schraudolph bf16-bits exp: bits16 = round(s*184.665 + 16250.5)
SCHRAU_MUL = 184.66496
SCHRAU_ADD = 16250.5
# group-1 kb index where the Act/DVE exp split starts
SCHRAU_KB0 = 14
# column split of late group-1 exp tiles: Act [0:ESPL], DVE [ESPL:1024]
ESPL = 832
# denominator accumulation column split: Pool [0:dspl], DVE [dspl:qw]
DSPL_A = 384
DSPL_B = 384

# cpak1 (stage-1): wtT 0:256 | wpT 256:512 | bt 512 | bp 513
CP1 = 514
# cpak2 (stage-2): wgT 0:256 | ident_bf 256:320 | bg 320 | bo 321:323
CP2 = 323
# cpak_cold (stage-3): woT 0:256 | ones 256:384
CPC = 384


def build():
    nc = bacc.Bacc(None, target_bir_lowering=False, debug=False)

    xb = nc.dram_tensor("xb", [CI, N], F32, kind="ExternalInput").ap()
    cpak1 = nc.dram_tensor("cpak1", [P, CP1], F32, kind="ExternalInput").ap()
    cpak2 = nc.dram_tensor("cpak2", [P, CP2], F32, kind="ExternalInput").ap()
    cpakc = nc.dram_tensor("cpakc", [P, CPC], F32, kind="ExternalInput").ap()
    oq = nc.dram_tensor("oq", [CO, Q], F32, kind="ExternalOutput").ap()

    with tile.TileContext(nc) as tc:
        with (
            tc.tile_pool(name="big", bufs=1) as big,
            tc.tile_pool(name="tmp", bufs=6) as tmp,
            tc.tile_pool(name="ps_s", bufs=2, space="PSUM") as ps_s,
            tc.tile_pool(name="ps_y", bufs=1, space="PSUM") as ps_y,
            tc.tile_pool(name="ps_proj", bufs=2, space="PSUM") as ps_proj,
        ):
            # ---- Act exp-table preload (dummy exp on a zeroed column) ----
            warm = big.tile([P, 1], F32, tag="warm")
            warm2 = big.tile([P, 1], F32, tag="warm2")
            nc.vector.memset(warm[:], 0.0)

            # ---- stage-1 DMAs (startup-critical): theta/phi weights + x0
            cpak1_sb = big.tile([P, CP1], MM_DT, tag="cpak1")
            nc.sync.dma_start(cpak1_sb[:], cpak1.bitcast(MM_DT))
            cpak2_sb = big.tile([P, CP2], MM_DT, tag="cpak2")
            cpakc_sb = big.tile([P, CPC], MM_DT, tag="cpakc")

            x_sb = big.tile([P, 2, N], MM_DT, tag="x")
            xbr = xb.rearrange("(o p) n -> p o n", p=P).bitcast(MM_DT)
            nc.scalar.dma_start(x_sb[:, :, 0:512], xbr[:, :, 0:512])
            # table load + warm exp queue behind the x0 issue on Act queue
            warm_inst = nc.scalar.activation(
                out=warm2[:], in_=warm[:], func=AF.Exp
            )
            # ---- stage-2 DMAs, gated on the warm exp (~1.3us after the
            # stage-1 issues) so stage-1 gets a bandwidth head start
            stage2 = [
                nc.sync.dma_start(cpak2_sb[:], cpak2.bitcast(MM_DT)),
                nc.sync.dma_start(x_sb[:, :, 512:1024], xbr[:, :, 512:1024]),
            ]
            for d in stage2:
                add_dep_helper(d.ins, warm_inst.ins, sync=True,
                               reason="stage-2 after stage-1 head start")

            wtT = cpak1_sb[:, 0:256].rearrange("p (o c) -> p o c", o=2)
            wpT = cpak1_sb[:, 256:512].rearrange("p (o c) -> p o c", o=2)
            bt_sb = cpak1_sb[:, 512:513].bitcast(F32)
            bp_sb = cpak1_sb[:, 513:514].bitcast(F32)
            wgT = cpak2_sb[:, 0:256].rearrange("p (o c) -> p o c", o=2)
            ident_bf = cpak2_sb[:, 256:320].bitcast(BF16)
            bg_sb = cpak2_sb[:, 320:321].bitcast(F32)
            bo_sb = cpak2_sb[:, 321:323].bitcast(F32)
            woT = cpakc_sb[:, 0:256].rearrange("p (o c) -> p o c", o=2)
            ones_sb = cpakc_sb[:, 256:384]

            # SBUF buffers shared across phases
            theta_sb = big.tile([P, Q], MM_DT, tag="theta")
            phi_sb = big.tile([P, N], MM_DT, tag="phi")
            g_sb = big.tile([P, N], BF16, tag="g")
            gT_sb = big.tile([P, N], BF16, tag="gT")
            y_sb = big.tile([P, Q], MM_DT, tag="y")
            d_acc = big.tile([P, Q], BF16, tag="dacc")
            d_f32 = big.tile([P, Q], MM_DT, tag="df32")
            out_sb = big.tile([P, 2, Q], F32, tag="out")
            oqr = oq.rearrange("(o p) q -> p o q", p=P)

            # ---- projection emitters --------------------------------------
            def proj(which, j, on_act=False):
                wT, bias, dst = {
                    "t": (wtT, bt_sb, theta_sb),
                    "p": (wpT, bp_sb, phi_sb),
                    "g": (wgT, bg_sb, g_sb),
                }[which]
                sl = slice(j * 512, (j + 1) * 512)
                pp = ps_proj.tile([P, 512], F32, tag="pp", name=f"pp{which}{j}")
                nc.tensor.matmul(
                    pp[:], wT[:, 0, :], x_sb[:, 0, sl], start=True, stop=False
                )
                nc.tensor.matmul(
                    pp[:], wT[:, 1, :], x_sb[:, 1, sl], start=False, stop=True
                )
                if on_act:
                    return nc.scalar.activation(
                        out=dst[:, sl], in_=pp[:], func=AF.Identity,
                        bias=bias,
                    )
                return nc.vector.tensor_scalar(
                    out=dst[:, sl], in0=pp[:],
                    scalar1=bias, scalar2=None, op0=OP.add,
                )

            def gtq(j, on_act=False):
                # transpose g columns 512j..512j+512 (4 key blocks, bf16 at
                # 1 cycle/row) into a bf16 bitcast slice of one proj-ring
                # PSUM tile, then one 2x-mode copy into gT
                sl = slice(j * 512, (j + 1) * 512)
                pq = ps_proj.tile([P, 512], F32, tag="pp", name=f"pq{j}")
                pqb = pq[:, 0:256].bitcast(BF16)
                for k in range(4):
                    ksl = slice(j * 512 + k * P, j * 512 + (k + 1) * P)
                    nc.tensor.transpose(
                        pqb[:, k * P : (k + 1) * P], g_sb[:, ksl], ident_bf
                    )
                if on_act:
                    nc.scalar.activation(
                        out=gT_sb[:, sl], in_=pqb[:], func=AF.Copy
                    )
                else:
                    nc.vector.tensor_copy(out=gT_sb[:, sl], in_=pqb[:])

            # ---- prologue: minimum for scores kb0 + y kb0 -----------------
            gate_inst = proj("t", 0, on_act=True)
            proj("p", 0, on_act=True)
            proj("t", 1, on_act=True)
            proj("g", 0, on_act=True)
            gtq(0, on_act=True)

            # ---- stage-3 DMAs: x tail + cold consts, gated on theta0 evac
            tail = [
                nc.sync.dma_start(x_sb[:, :, 1024:2048], xbr[:, :, 1024:2048]),
                nc.sync.dma_start(x_sb[:, :, 2048:3072], xbr[:, :, 2048:3072]),
                nc.sync.dma_start(x_sb[:, :, 3072:4096], xbr[:, :, 3072:4096]),
                nc.sync.dma_start(cpakc_sb[:], cpakc.bitcast(MM_DT)),
            ]
            for d in tail:
                add_dep_helper(d.ins, gate_inst.ins, sync=True,
                               reason="x tail after startup-critical DMAs")

            # deferred projection pieces, one per kb of group 0.  phi j
            # feeds scores kb=4j (emitted one kb early); gT quad j feeds y
            # kb=4j; theta j2/j3 feed group 1.
            work = [
                (proj, ("p", 1)), (proj, ("g", 1)), (proj, ("t", 2)),
                (gtq, (1,)),
            ]
            for j in range(2, 8):
                work += [(proj, ("p", j)), (proj, ("g", j)), (gtq, (j,))]
            work += [(proj, ("t", 3))]

            def exp_pieces(gi, kb, final):
                """[(engine, c0, c1)] for the exp of tile (gi, kb)."""
                if gi == 0:
                    if kb < 2:
                        return [("act", 0, 512), ("act", 512, 1024)]
                    return [("act", 0, 1024)]
                if final and kb >= NKB - 2:
                    return [("act", 0, 512), ("act", 512, 1024)]
                if kb >= SCHRAU_KB0:
                    return [("act", 0, ESPL), ("dve", ESPL, 1024)]
                return [("act", 0, 1024)]

            def attn_group(gi, q0, qw, work=None, pending_out=None,
                           final=False):
                qsl = slice(q0, q0 + qw)
                nh = qw // 512
                with nc.named_scope(f"attn{gi}"):
                    y_ps = ps_y.tile([P, qw], F32, tag="y", name=f"y_ps{gi}")

                    def scores(kb):
                        s_ps = ps_s.tile(
                            [P, qw], F32, tag="s", name=f"s{gi}_{kb}"
                        )
                        for h in range(nh):
                            nc.tensor.matmul(
                                s_ps[:, h * 512 : (h + 1) * 512],
                                phi_sb[:, kb * P : (kb + 1) * P],
                                theta_sb[:, q0 + h * 512 : q0 + (h + 1) * 512],
                                start=True, stop=True,
                            )
                        return s_ps

                    s_cur = scores(0)
                    pend_at = []
                    dp_scr = [None]
                    dinit = [True]
                    for kb in range(NKB):
                        at = tmp.tile(
                            [P, qw], BF16, tag="attn", name=f"at{gi}_{kb}"
                        )
                        for eng, c0, c1 in exp_pieces(gi, kb, final):
                            if eng == "act":
                                nc.scalar.activation(
                                    out=at[:, c0:c1], in_=s_cur[:, c0:c1],
                                    func=AF.Exp,
                                )
                            else:
                                nc.vector.tensor_scalar(
                                    out=at[:, c0:c1].bitcast(U16),
                                    in0=s_cur[:, c0:c1],
                                    scalar1=SCHRAU_MUL, scalar2=SCHRAU_ADD,
                                    op0=OP.mult, op1=OP.add,
                                )
                        if kb + 1 < NKB:
                            # feed the PE the next scores before y(kb) so it
                            # is not idle while exp(kb) runs
                            s_cur = scores(kb + 1)
                        # one deferred projection piece per kb (group 0)
                        if work:
                            fn, arg = work.pop(0)
                            fn(*arg)
                        first, last = kb == 0, kb == NKB - 1
                        for h in range(nh):
                            hsl = slice(h * 512, (h + 1) * 512)
                            nc.tensor.matmul(
                                y_ps[:, hsl],
                                gT_sb[:, kb * P : (kb + 1) * P],
                                at[:, hsl], start=first, stop=last,
                            )
                        # softmax denominator on DVE, all ops FULL-TILE (the
                        # 2x hw mode engages only on full-tile operands) and
                        # deferred so the queue never stalls: at tiles are
                        # summed in PAIRS (d_pair = at_a + at_b, then
                        # d_acc += d_pair) -- amortized one 597ns op per kb
                        # instead of one 692ns accumulate per kb.  The final
                        # kb is accumulated immediately, split per half, so
                        # the h0 epilogue overlaps the h1 exp.
                        def flush_dpair():
                            if dp_scr[0] is not None:
                                nc.vector.tensor_tensor(
                                    out=d_acc[:, qsl], in0=d_acc[:, qsl],
                                    in1=dp_scr[0][:], op=OP.add,
                                )
                                dp_scr[0] = None

                        def pair_pend():
                            a0, a1 = pend_at
                            if dinit[0]:
                                nc.vector.tensor_tensor(
                                    out=d_acc[:, qsl], in0=a0[:], in1=a1[:],
                                    op=OP.add,
                                )
                                dinit[0] = False
                            else:
                                dp = tmp.tile(
                                    [P, qw], BF16, tag="dpair",
                                    name=f"dp{gi}_{kb}",
                                )
                                nc.vector.tensor_tensor(
                                    out=dp[:], in0=a0[:], in1=a1[:],
                                    op=OP.add,
                                )
                                dp_scr[0] = dp
                            pend_at.clear()

                        if final and kb == NKB - 1:
                            # flush all pending state, then at(31) per half
                            flush_dpair()
                            if len(pend_at) == 2:
                                pair_pend()
                                flush_dpair()
                            elif len(pend_at) == 1:
                                nc.vector.tensor_tensor(
                                    out=d_acc[:, qsl], in0=d_acc[:, qsl],
                                    in1=pend_at.pop()[:], op=OP.add,
                                )
                            for c0, c1 in ((0, 512), (512, 1024)):
                                dsl = slice(q0 + c0, q0 + c1)
                                nc.vector.tensor_tensor(
                                    out=d_acc[:, dsl], in0=d_acc[:, dsl],
                                    in1=at[:, c0:c1], op=OP.add,
                                )
                        else:
                            if dp_scr[0] is not None:
                                flush_dpair()
                            elif len(pend_at) == 2:
                                pair_pend()
                            pend_at.append(at)
                        # previous group's output projection, one piece per
                        # kb starting at kb=2
                        if pending_out and kb >= 2 and kb % 2 == 0:
                            pending_out.pop(0)()
                    flush_dpair()
                    if len(pend_at) == 2:
                        pair_pend()
                        flush_dpair()
                    elif len(pend_at) == 1:
                        nc.vector.tensor_tensor(
                            out=d_acc[:, qsl], in0=d_acc[:, qsl],
                            in1=pend_at.pop()[:], op=OP.add,
                        )
                    while pending_out:
                        pending_out.pop(0)()
                    # evacuate y: DVE for group 0 (Act is streaming exps for
                    # group 1 then); Act for the final group (it is idle)
                    for h in range(nh):
                        hsl = slice(h * 512, (h + 1) * 512)
                        qhsl = slice(q0 + h * 512, q0 + (h + 1) * 512)
                        nc.scalar.activation(
                            out=y_sb[:, qhsl], in_=y_ps[:, hsl],
                            func=AF.Copy,
                        )

                pieces = []
                rd = big.tile([P, qw], F32, tag=f"rd{gi}")

                def epi_h(h):
                    # per-512 epilogue: bf16->f32r denominator cast, ones
                    # matmul partition-reduce+broadcast, reciprocal
                    hsl = slice(h * 512, (h + 1) * 512)
                    qhsl = slice(q0 + h * 512, q0 + (h + 1) * 512)
                    with nc.named_scope(f"epi{gi}"):
                        d_bc = ps_proj.tile(
                            [P, 512], F32, tag="pp", name=f"dbc{gi}{h}"
                        )
                        nc.vector.tensor_copy(
                            out=d_f32[:, qhsl], in_=d_acc[:, qhsl]
                        )
                        nc.tensor.matmul(
                            d_bc[:], ones_sb, d_f32[:, qhsl],
                            start=True, stop=True,
                        )
                        nc.vector.reciprocal_approx_fast(
                            out=rd[:, hsl], in_=d_bc[:],
                        )

                def out_piece(blk, h):
                    hsl = slice(h * 512, (h + 1) * 512)
                    qhsl = slice(q0 + h * 512, q0 + (h + 1) * 512)
                    with nc.named_scope(f"outp{gi}"):
                        po = ps_proj.tile(
                            [P, 512], F32, tag="pp", name=f"po{gi}{blk}{h}"
                        )
                        nc.tensor.matmul(
                            po[:], woT[:, blk, :], y_sb[:, qhsl],
                            start=True, stop=True,
                        )
                        # out = (po + b_out) * (1/d) in one DVE pass
                        nc.vector.scalar_tensor_tensor(
                            out=out_sb[:, blk, qhsl], in0=po[:],
                            scalar=bo_sb[:, blk : blk + 1], in1=rd[:, hsl],
                            op0=OP.add, op1=OP.mult,
                        )
                        nc.sync.dma_start(
                            oqr[:, blk, qhsl], out_sb[:, blk, qhsl]
                        )

                # h-major so each half's chain drains independently
                for h in range(nh):
                    pieces.append(lambda h=h: epi_h(h))
                    for blk in range(2):
                        pieces.append(lambda blk=blk, h=h: out_piece(blk, h))
                if final:
                    for p in pieces:
                        p()
                    return []
                return pieces

            out0 = attn_group(0, 0, 1024, work=work)
            attn_group(1, 1024, 1024, pending_out=out0, final=True)

    nc.compile()
    return nc


_NC_CACHE = None
LAST_EXEC_TIME_NS = None
LAST_TRACE = None
LAST_RESULTS = None


def _get_nc():
    global _NC_CACHE
    if _NC_CACHE is None:
        _NC_CACHE = build()
    return _NC_CACHE


def kernel(**inputs):
    x = np.ascontiguousarray(np.asarray(inputs["x"], dtype=np.float32))
    assert x.shape == (B, CI, T, H, W), x.shape
    xf = x.reshape(B, CI, N)
    w = {
        k: np.ascontiguousarray(np.asarray(inputs[k], dtype=np.float32))
        for k in (
            "w_theta", "b_theta", "w_phi", "b_phi", "w_g", "b_g", "w_out",
            "b_out",
        )
    }

    def proj_t(wm):
        # [p, o*128+c] = wm[c, o*128+p]
        return wm.T.reshape(2, P, P).transpose(1, 0, 2).reshape(P, 2 * P)

    woT_h = w["w_out"].reshape(2, P, CINT).transpose(2, 0, 1).reshape(P, 2 * P)
    # bf16 identity packed into 64 f32 columns
    i16 = np.zeros((P, P), np.uint32)
    i16[np.arange(P), np.arange(P)] = 0x3F80
    ident_bf_packed = (i16[:, 0::2] | (i16[:, 1::2] << 16)).view(np.float32)
    CPAK1 = np.ascontiguousarray(
        np.concatenate(
            [
                proj_t(w["w_theta"]), proj_t(w["w_phi"]),
                w["b_theta"][:, None], w["b_phi"][:, None],
            ],
            axis=1,
        )
    )
    assert CPAK1.shape == (P, CP1), CPAK1.shape
    CPAK2 = np.ascontiguousarray(
        np.concatenate(
            [
                proj_t(w["w_g"]), ident_bf_packed,
                w["b_g"][:, None],
                w["b_out"][:P, None], w["b_out"][P:, None],
            ],
            axis=1,
        )
    )
    assert CPAK2.shape == (P, CP2), CPAK2.shape
    CPAKC = np.ascontiguousarray(
        np.concatenate([woT_h, np.ones((P, P), np.float32)], axis=1)
    )
    assert CPAKC.shape == (P, CPC), CPAKC.shape

    in_maps = []
    for core in range(8):
        b, h = core // 2, core % 2
        if h == 0:
            xcore = xf[b]
        else:
            xcore = np.ascontiguousarray(
                np.concatenate([xf[b][:, Q:], xf[b][:, :Q]], axis=1)
            )
        in_maps.append(
            {"xb": xcore, "cpak1": CPAK1, "cpak2": CPAK2, "cpakc": CPAKC}
        )

    nc = _get_nc()
    res = run_bass_kernel_spmd(nc, in_maps, core_ids=list(range(8)))
    global LAST_EXEC_TIME_NS, LAST_TRACE, LAST_RESULTS
    LAST_EXEC_TIME_NS = res.exec_time_ns
    LAST_TRACE = (
        res.instructions_and_trace[1] if res.instructions_and_trace else None
    )
    LAST_RESULTS = res

    out = np.empty((B, CO, N), np.float32)
    for core in range(8):
        b, h = core // 2, core % 2
        out[b][:, h * Q : (h + 1) * Q] = res.results[core]["oq"]
    return out.reshape(B, CO, T, H, W)


# revision 9
# speedup vs baseline: 1.0084x; 1.0084x over previous
"""Non-local block (B=4, C_in=256, C_int=128, C_out=256, N=T*H*W=4096) on 8
Trainium2 NeuronCores.

Sharding: data-parallel over batch (4 batches) x query-halves (2) = 8 cores.
Each core holds one batch's full x (for keys/values); the host rotates each
core's columns so its 2048 queries are always columns 0:2048 (attention is
permutation-invariant over keys). Per core: theta/phi/g projections, the
[2048q x 4096k] attention with softmax (keys on partitions), and the output
projection for its query half. Host gathers the 8 [256, 2048] slices.

v2 engine layout:
  PE:   scores + y + projections + bf16 gT transposes + denominator
        broadcast (ones matmul).  Single PSUM scope for the whole kernel
        (scores ring 2x[128,1024]=4 banks, y 1x[128,1024]=2, proj/piece ring
        2x[128,512]=2) so there is no pool-boundary drain between the two
        query groups.
  Act:  pure exp stream (plus final-group y evac).  Table pre-loaded via a
        dummy exp during the DMA wait.
  DVE:  PSUM evacuations (theta/phi f32r, g bf16, gT copy, y f32r), the
        high columns of the softmax denominator accumulation, the epilogue
        (cast/recip/scale), and a Schraudolph approximate-exp share of the
        late group-1 tiles (one tensor_scalar: bf16 bits = s*184.665+16250.5
        converted to uint16).
  Pool (gpsimd): low columns of the denominator accumulation
        (SBUF-only; GPSIMD cannot touch PSUM).
  DMA:  staged; the x tail + cold constants are gated behind the first
        theta evacuation so the startup-critical cpak_hot+x0 transfers get
        full HBM bandwidth.
"""

import sys
import types

import numpy as np

import concourse.bacc as bacc
import concourse.mybir as mybir
import concourse.tile as tile
from concourse.bass_utils import run_bass_kernel_spmd
from concourse.tile import add_dep_helper


def _install_ntff_hook():
    try:
        import antenv.axon_hooks  # noqa: F401
        return
    except ImportError:
        pass
    try:
        from trn_agent_boot.trn_boot import _ntff_profile_via_ctypes

        hook = _ntff_profile_via_ctypes("/opt/axon/libaxon_pjrt.so")
    except Exception:
        hook = None
    mod = types.ModuleType("antenv.axon_hooks")
    mod.get_axon_ntff_profile_hook = lambda: hook
    mod.set_axon_ntff_profile_hook = lambda h: None
    sys.modules["antenv.axon_hooks"] = mod


_install_ntff_hook()

F32 = mybir.dt.float32
F32R = mybir.dt.float32r
BF16 = mybir.dt.bfloat16
U16 = mybir.dt.uint16
AF = mybir.ActivationFunctionType
OP = mybir.AluOpType

P = 128
CI = 256  # input channels (2 chunks of 128)
CINT = 128  # intermediate channels
CO = 256  # output channels (2 blocks of 128)
N = 4096  # key/value positions (32 blocks of 128)
Q = 2048  # queries per core
B, T, H, W = 4, 4, 32, 32
NKB = N // P  # 32 key blocks

MM_DT = F32R

# Schraudolph bf16-bits exp: bits16 = round(s*184.665 + 16250.5)
SCHRAU_MUL = 184.66496
SCHRAU_ADD = 16250.5
# group-1 kb index where the Act/DVE exp split starts
SCHRAU_KB0 = 14
# column split of late group-1 exp tiles: Act [0:ESPL], DVE [ESPL:1024]
ESPL = 896
# denominator accumulation column split: Pool [0:dspl], DVE [dspl:qw]
DSPL_A = 384
DSPL_B = 384

# cpak1 (stage-1): wtT 0:256 | wpT 256:512 | bt 512 | bp 513
CP1 = 514
# cpak2 (stage-2): wgT 0:256 | ident_bf 256:320 | bg 320 | bo 321:323
CP2 = 323
# cpak_cold (stage-3): woT 0:256 | ones 256:384
CPC = 384


def build():
    nc = bacc.Bacc(None, target_bir_lowering=False, debug=False)

    xb = nc.dram_tensor("xb", [CI, N], F32, kind="ExternalInput").ap()
    cpak1 = nc.dram_tensor("cpak1", [P, CP1], F32, kind="ExternalInput").ap()
    cpak2 = nc.dram_tensor("cpak2", [P, CP2], F32, kind="ExternalInput").ap()
    cpakc = nc.dram_tensor("cpakc", [P, CPC], F32, kind="ExternalInput").ap()
    oq = nc.dram_tensor("oq", [CO, Q], F32, kind="ExternalOutput").ap()

    with tile.TileContext(nc) as tc:
        with (
            tc.tile_pool(name="big", bufs=1) as big,
            tc.tile_pool(name="tmp", bufs=6) as tmp,
            tc.tile_pool(name="ps_s", bufs=2, space="PSUM") as ps_s,
            tc.tile_pool(name="ps_y", bufs=1, space="PSUM") as ps_y,
            tc.tile_pool(name="ps_proj", bufs=2, space="PSUM") as ps_proj,
        ):
            # ---- Act exp-table preload (dummy exp on a zeroed column) ----
            warm = big.tile([P, 1], F32, tag="warm")
            warm2 = big.tile([P, 1], F32, tag="warm2")
            nc.vector.memset(warm[:], 0.0)

            # ---- stage-1 DMAs (startup-critical): theta/phi weights + x0
            cpak1_sb = big.tile([P, CP1], MM_DT, tag="cpak1")
            nc.sync.dma_start(cpak1_sb[:], cpak1.bitcast(MM_DT))
            cpak2_sb = big.tile([P, CP2], MM_DT, tag="cpak2")
            cpakc_sb = big.tile([P, CPC], MM_DT, tag="cpakc")

            x_sb = big.tile([P, 2, N], MM_DT, tag="x")
            xbr = xb.rearrange("(o p) n -> p o n", p=P).bitcast(MM_DT)
            nc.scalar.dma_start(x_sb[:, :, 0:512], xbr[:, :, 0:512])
            # table load + warm exp queue behind the x0 issue on Act queue
            warm_inst = nc.scalar.activation(
                out=warm2[:], in_=warm[:], func=AF.Exp
            )
            # ---- stage-2 DMAs, gated on the warm exp (~1.3us after the
            # stage-1 issues) so stage-1 gets a bandwidth head start
            stage2 = [
                nc.sync.dma_start(cpak2_sb[:], cpak2.bitcast(MM_DT)),
                nc.sync.dma_start(x_sb[:, :, 512:1024], xbr[:, :, 512:1024]),
            ]
            for d in stage2:
                add_dep_helper(d.ins, warm_inst.ins, sync=True,
                               reason="stage-2 after stage-1 head start")

            wtT = cpak1_sb[:, 0:256].rearrange("p (o c) -> p o c", o=2)
            wpT = cpak1_sb[:, 256:512].rearrange("p (o c) -> p o c", o=2)
            bt_sb = cpak1_sb[:, 512:513].bitcast(F32)
            bp_sb = cpak1_sb[:, 513:514].bitcast(F32)
            wgT = cpak2_sb[:, 0:256].rearrange("p (o c) -> p o c", o=2)
            ident_bf = cpak2_sb[:, 256:320].bitcast(BF16)
            bg_sb = cpak2_sb[:, 320:321].bitcast(F32)
            bo_sb = cpak2_sb[:, 321:323].bitcast(F32)
            woT = cpakc_sb[:, 0:256].rearrange("p (o c) -> p o c", o=2)
            ones_sb = cpakc_sb[:, 256:384]

            # SBUF buffers shared across phases
            theta_sb = big.tile([P, Q], MM_DT, tag="theta")
            phi_sb = big.tile([P, N], MM_DT, tag="phi")
            g_sb = big.tile([P, N], BF16, tag="g")
            gT_sb = big.tile([P, N], BF16, tag="gT")
            y_sb = big.tile([P, Q], MM_DT, tag="y")
            d_acc = big.tile([P, Q], BF16, tag="dacc")
            d_f32 = big.tile([P, Q], MM_DT, tag="df32")
            out_sb = big.tile([P, 2, Q], F32, tag="out")
            oqr = oq.rearrange("(o p) q -> p o q", p=P)

            # ---- projection emitters --------------------------------------
            def proj(which, j, on_act=False):
                wT, bias, dst = {
                    "t": (wtT, bt_sb, theta_sb),
                    "p": (wpT, bp_sb, phi_sb),
                    "g": (wgT, bg_sb, g_sb),
                }[which]
                sl = slice(j * 512, (j + 1) * 512)
                pp = ps_proj.tile([P, 512], F32, tag="pp", name=f"pp{which}{j}")
                nc.tensor.matmul(
                    pp[:], wT[:, 0, :], x_sb[:, 0, sl], start=True, stop=False
                )
                nc.tensor.matmul(
                    pp[:], wT[:, 1, :], x_sb[:, 1, sl], start=False, stop=True
                )
                if on_act:
                    return nc.scalar.activation(
                        out=dst[:, sl], in_=pp[:], func=AF.Identity,
                        bias=bias,
                    )
                return nc.vector.tensor_scalar(
                    out=dst[:, sl], in0=pp[:],
                    scalar1=bias, scalar2=None, op0=OP.add,
                )

            def gtq(j, on_act=False):
                # transpose g columns 512j..512j+512 (4 key blocks, bf16 at
                # 1 cycle/row) into a bf16 bitcast slice of one proj-ring
                # PSUM tile, then one 2x-mode copy into gT
                sl = slice(j * 512, (j + 1) * 512)
                pq = ps_proj.tile([P, 512], F32, tag="pp", name=f"pq{j}")
                pqb = pq[:, 0:256].bitcast(BF16)
                for k in range(4):
                    ksl = slice(j * 512 + k * P, j * 512 + (k + 1) * P)
                    nc.tensor.transpose(
                        pqb[:, k * P : (k + 1) * P], g_sb[:, ksl], ident_bf
                    )
                if on_act:
                    nc.scalar.activation(
                        out=gT_sb[:, sl], in_=pqb[:], func=AF.Copy
                    )
                else:
                    nc.vector.tensor_copy(out=gT_sb[:, sl], in_=pqb[:])

            # ---- prologue: minimum for scores kb0 + y kb0 -----------------
            gate_inst = proj("t", 0, on_act=True)
            proj("p", 0, on_act=True)
            proj("t", 1, on_act=True)
            proj("g", 0, on_act=True)
            gtq(0, on_act=True)

            # ---- stage-3 DMAs: x tail + cold consts, gated on theta0 evac
            tail = [
                nc.sync.dma_start(x_sb[:, :, 1024:2048], xbr[:, :, 1024:2048]),
                nc.sync.dma_start(x_sb[:, :, 2048:3072], xbr[:, :, 2048:3072]),
                nc.sync.dma_start(x_sb[:, :, 3072:4096], xbr[:, :, 3072:4096]),
                nc.sync.dma_start(cpakc_sb[:], cpakc.bitcast(MM_DT)),
            ]
            for d in tail:
                add_dep_helper(d.ins, gate_inst.ins, sync=True,
                               reason="x tail after startup-critical DMAs")

            # deferred projection pieces, one per kb of group 0.  phi j
            # feeds scores kb=4j (emitted one kb early); gT quad j feeds y
            # kb=4j; theta j2/j3 feed group 1.
            work = [
                (proj, ("p", 1)), (proj, ("g", 1)), (proj, ("t", 2)),
                (gtq, (1,)),
            ]
            for j in range(2, 8):
                work += [(proj, ("p", j)), (proj, ("g", j)), (gtq, (j,))]
            work += [(proj, ("t", 3))]

            def exp_pieces(gi, kb, final):
                """[(engine, c0, c1)] for the exp of tile (gi, kb)."""
                if gi == 0:
                    if kb < 2:
                        return [("act", 0, 512), ("act", 512, 1024)]
                    return [("act", 0, 1024)]
                if final and kb >= NKB - 2:
                    return [("act", 0, 512), ("act", 512, 1024)]
                if kb >= SCHRAU_KB0:
                    return [("act", 0, ESPL), ("dve", ESPL, 1024)]
                return [("act", 0, 1024)]

            def attn_group(gi, q0, qw, work=None, pending_out=None,
                           final=False):
                qsl = slice(q0, q0 + qw)
                nh = qw // 512
                with nc.named_scope(f"attn{gi}"):
                    y_ps = ps_y.tile([P, qw], F32, tag="y", name=f"y_ps{gi}")

                    def scores(kb):
                        s_ps = ps_s.tile(
                            [P, qw], F32, tag="s", name=f"s{gi}_{kb}"
                        )
                        for h in range(nh):
                            nc.tensor.matmul(
                                s_ps[:, h * 512 : (h + 1) * 512],
                                phi_sb[:, kb * P : (kb + 1) * P],
                                theta_sb[:, q0 + h * 512 : q0 + (h + 1) * 512],
                                start=True, stop=True,
                            )
                        return s_ps

                    s_cur = scores(0)
                    pend_at = [None]
                    for kb in range(NKB):
                        at = tmp.tile(
                            [P, qw], BF16, tag="attn", name=f"at{gi}_{kb}"
                        )
                        for eng, c0, c1 in exp_pieces(gi, kb, final):
                            if eng == "act":
                                nc.scalar.activation(
                                    out=at[:, c0:c1], in_=s_cur[:, c0:c1],
                                    func=AF.Exp,
                                )
                            else:
                                nc.vector.tensor_scalar(
                                    out=at[:, c0:c1].bitcast(U16),
                                    in0=s_cur[:, c0:c1],
                                    scalar1=SCHRAU_MUL, scalar2=SCHRAU_ADD,
                                    op0=OP.mult, op1=OP.add,
                                )
                        if kb + 1 < NKB:
                            # feed the PE the next scores before y(kb) so it
                            # is not idle while exp(kb) runs
                            s_cur = scores(kb + 1)
                        # one deferred projection piece per kb (group 0)
                        if work:
                            fn, arg = work.pop(0)
                            fn(*arg)
                        first, last = kb == 0, kb == NKB - 1
                        for h in range(nh):
                            hsl = slice(h * 512, (h + 1) * 512)
                            nc.tensor.matmul(
                                y_ps[:, hsl],
                                gT_sb[:, kb * P : (kb + 1) * P],
                                at[:, hsl], start=first, stop=last,
                            )
                        # softmax denominator: full-tile DVE adds (the 2x
                        # hw mode engages only on full-tile ops), DEFERRED
                        # one iteration so the DVE queue never stalls: at
                        # iter kb it accumulates at(kb-1), which Act/DVE
                        # finished writing last iteration.  Final kb runs
                        # immediately, split per half, so the h0 epilogue
                        # overlaps the h1 exp.
                        def dacc(a, c0, c1):
                            dsl = slice(q0 + c0, q0 + c1)
                            if a[1]:
                                nc.vector.tensor_copy(
                                    out=d_acc[:, dsl], in_=a[0][:, c0:c1]
                                )
                            else:
                                nc.vector.tensor_tensor(
                                    out=d_acc[:, dsl], in0=d_acc[:, dsl],
                                    in1=a[0][:, c0:c1], op=OP.add,
                                )

                        if pend_at[0] is not None:
                            dacc(pend_at[0], 0, qw)
                        pend_at[0] = (at, kb == 0)
                        if final and kb == NKB - 1:
                            dacc(pend_at[0], 0, 512)
                            dacc(pend_at[0], 512, 1024)
                            pend_at[0] = None
                        # previous group's output projection, one piece per
                        # kb starting at kb=2
                        if pending_out and kb >= 2 and kb % 2 == 0:
                            pending_out.pop(0)()
                    if pend_at[0] is not None:
                        dacc(pend_at[0], 0, qw)
                        pend_at[0] = None
                    while pending_out:
                        pending_out.pop(0)()
                    # evacuate y: DVE for group 0 (Act is streaming exps for
                    # group 1 then); Act for the final group (it is idle)
                    for h in range(nh):
                        hsl = slice(h * 512, (h + 1) * 512)
                        qhsl = slice(q0 + h * 512, q0 + (h + 1) * 512)
                        nc.scalar.activation(
                            out=y_sb[:, qhsl], in_=y_ps[:, hsl],
                            func=AF.Copy,
                        )

                pieces = []
                rd = big.tile([P, qw], F32, tag=f"rd{gi}")

                def epi_h(h):
                    # per-512 epilogue: bf16->f32r denominator cast, ones
                    # matmul partition-reduce+broadcast, reciprocal
                    hsl = slice(h * 512, (h + 1) * 512)
                    qhsl = slice(q0 + h * 512, q0 + (h + 1) * 512)
                    with nc.named_scope(f"epi{gi}"):
                        d_bc = ps_proj.tile(
                            [P, 512], F32, tag="pp", name=f"dbc{gi}{h}"
                        )
                        nc.vector.tensor_copy(
                            out=d_f32[:, qhsl], in_=d_acc[:, qhsl]
                        )
                        nc.tensor.matmul(
                            d_bc[:], ones_sb, d_f32[:, qhsl],
                            start=True, stop=True,
                        )
                        nc.vector.reciprocal_approx_fast(
                            out=rd[:, hsl], in_=d_bc[:],
                        )

                def out_piece(blk, h):
                    hsl = slice(h * 512, (h + 1) * 512)
                    qhsl = slice(q0 + h * 512, q0 + (h + 1) * 512)
                    with nc.named_scope(f"outp{gi}"):
                        po = ps_proj.tile(
                            [P, 512], F32, tag="pp", name=f"po{gi}{blk}{h}"
                        )
                        nc.tensor.matmul(
                            po[:], woT[:, blk, :], y_sb[:, qhsl],
                            start=True, stop=True,
                        )
                        # out = (po + b_out) * (1/d) in one DVE pass
                        nc.vector.scalar_tensor_tensor(
                            out=out_sb[:, blk, qhsl], in0=po[:],
                            scalar=bo_sb[:, blk : blk + 1], in1=rd[:, hsl],
                            op0=OP.add, op1=OP.mult,
                        )
                        nc.sync.dma_start(
                            oqr[:, blk, qhsl], out_sb[:, blk, qhsl]
                        )

                # h-major so each half's chain drains independently
                for h in range(nh):
                    pieces.append(lambda h=h: epi_h(h))
                    for blk in range(2):
                        pieces.append(lambda blk=blk, h=h: out_piece(blk, h))
                if final:
                    for p in pieces:
                        p()
                    return []
                return pieces

            out0 = attn_group(0, 0, 1024, work=work)
            attn_group(1, 1024, 1024, pending_out=out0, final=True)

    nc.compile()
    return nc


_NC_CACHE = None
LAST_EXEC_TIME_NS = None
LAST_TRACE = None
LAST_RESULTS = None


def _get_nc():
    global _NC_CACHE
    if _NC_CACHE is None:
        _NC_CACHE = build()
    return _NC_CACHE


def kernel(**inputs):
    x = np.ascontiguousarray(np.asarray(inputs["x"], dtype=np.float32))
    assert x.shape == (B, CI, T, H, W), x.shape
    xf = x.reshape(B, CI, N)
    w = {
        k: np.ascontiguousarray(np.asarray(inputs[k], dtype=np.float32))
        for k in (
            "w_theta", "b_theta", "w_phi", "b_phi", "w_g", "b_g", "w_out",
            "b_out",
        )
    }

    def proj_t(wm):
        # [p, o*128+c] = wm[c, o*128+p]
        return wm.T.reshape(2, P, P).transpose(1, 0, 2).reshape(P, 2 * P)

    woT_h = w["w_out"].reshape(2, P, CINT).transpose(2, 0, 1).reshape(P, 2 * P)
    # bf16 identity packed into 64 f32 columns
    i16 = np.zeros((P, P), np.uint32)
    i16[np.arange(P), np.arange(P)] = 0x3F80
    ident_bf_packed = (i16[:, 0::2] | (i16[:, 1::2] << 16)).view(np.float32)
    CPAK1 = np.ascontiguousarray(
        np.concatenate(
            [
                proj_t(w["w_theta"]), proj_t(w["w_phi"]),
                w["b_theta"][:, None], w["b_phi"][:, None],
            ],
            axis=1,
        )
    )
    assert CPAK1.shape == (P, CP1), CPAK1.shape
    CPAK2 = np.ascontiguousarray(
        np.concatenate(
            [
                proj_t(w["w_g"]), ident_bf_packed,
                w["b_g"][:, None],
                w["b_out"][:P, None], w["b_out"][P:, None],
            ],
            axis=1,
        )
    )
    assert CPAK2.shape == (P, CP2), CPAK2.shape
    CPAKC = np.ascontiguousarray(
        np.concatenate([woT_h, np.ones((P, P), np.float32)], axis=1)
    )
    assert CPAKC.shape == (P, CPC), CPAKC.shape

    in_maps = []
    for core in range(8):
        b, h = core // 2, core % 2
        if h == 0:
            xcore = xf[b]
        else:
            xcore = np.ascontiguousarray(
                np.concatenate([xf[b][:, Q:], xf[b][:, :Q]], axis=1)
            )
        in_maps.append(
            {"xb": xcore, "cpak1": CPAK1, "cpak2": CPAK2, "cpakc": CPAKC}
        )

    nc = _get_nc()
    res = run_bass_kernel_spmd(nc, in_maps, core_ids=list(range(8)))
    global LAST_EXEC_TIME_NS, LAST_TRACE, LAST_RESULTS
    LAST_EXEC_TIME_NS = res.exec_time_ns
    LAST_TRACE = (
        res.instructions_and_trace[1] if res.instructions_and_trace else None
    )
    LAST_RESULTS = res

    out = np.empty((B, CO, N), np.float32)
    for core in range(8):
        b, h = core // 2, core % 2
        out[b][:, h * Q : (h + 1) * Q] = res.results[core]["oq"]
    return out.reshape(B, CO, T, H, W)
